# revision 9
# baseline (speedup 1.0000x reference)
"""MFA block kernel for 8 Trainium2 NeuronCores.

Full (unsharded) inputs in, full output out. Internally shards the
flattened token dim (8*1024 = 8192) across 8 cores (1024 tokens each) and
uses the associative rewrite

    y = theta_x @ (phi_x^T @ g_x) / BN

so the (BN, BN) attention matrix is never formed.  With the homogeneous
trick X_ext = [x_l | 1], the per-core contribution to M = phi_x^T @ g_x is

    M_local = P_ext^T (X_ext^T X_ext) G_ext,   P_ext = [phi_w; phi_b],
                                               G_ext = [g_w; g_b],

so only C_ext = X_ext^T X_ext (257x257) needs an AllReduce.  BatchNorm in
training mode needs global per-feature sum / sum-of-squares of
w_y' = theta_x @ (M/BN) @ w_w  -- a second, tiny (2x512) AllReduce.
(w_b is dropped: BN output is invariant to constant shifts.)  The final
output is recomputed on the (otherwise idle) PE as

    z = theta_ext @ [V*A; B] + x_h,   A = gamma * rsqrt(var+eps),
                                      B = beta - mean*A,

where theta_ext = [theta_x | 1] lives feature-major in SBUF.
"""

import threading

import numpy as np

import concourse.bass as bass
import concourse.tile as tile
from concourse import bacc, mybir
from concourse.bass_utils import run_bass_kernel_spmd
from concourse.masks import make_identity

FP = mybir.dt.float32
HIGH = 512
LOW = 256
B = 8
N = 1024
BN = B * N            # 8192 flattened tokens
NCORES = 8
TPC = BN // NCORES    # 1024 tokens per core
TT = TPC // 128       # 8 token tiles per core
EPS = 1e-5

LOWE = LOW + 1        # 257: homogeneous low dim


def _mslice(mc):
    """m-chunk slices over the 257-long extended-low dim."""
    return (slice(0, 128), slice(128, 256), slice(256, 257))[mc]


def build_kernel(repeats: int = 1):
    nc = bacc.Bacc("TRN2", target_bir_lowering=False, debug=False,
                   num_devices=NCORES)

    x_h = nc.declare_dram_parameter("x_h", [TPC, HIGH], FP, isOutput=False)
    x_l = nc.declare_dram_parameter("x_l", [TPC, LOW], FP, isOutput=False)
    g_w = nc.declare_dram_parameter("g_w", [LOW, LOW], FP, isOutput=False)
    g_b = nc.declare_dram_parameter("g_b", [LOW], FP, isOutput=False)
    theta_w = nc.declare_dram_parameter("theta_w", [HIGH, LOW], FP, isOutput=False)
    theta_b = nc.declare_dram_parameter("theta_b", [LOW], FP, isOutput=False)
    phi_w = nc.declare_dram_parameter("phi_w", [LOW, LOW], FP, isOutput=False)
    phi_b = nc.declare_dram_parameter("phi_b", [LOW], FP, isOutput=False)
    w_w = nc.declare_dram_parameter("w_w", [LOW, HIGH], FP, isOutput=False)
    bn_gamma = nc.declare_dram_parameter("bn_gamma", [HIGH], FP, isOutput=False)
    bn_beta = nc.declare_dram_parameter("bn_beta", [HIGH], FP, isOutput=False)
    z_out = nc.declare_dram_parameter("z", [TPC, HIGH], FP, isOutput=True)

    rg = [list(range(NCORES))]

    with tile.TileContext(nc) as tc:
        with (
            tc.tile_pool(name="sb", bufs=1) as sb,
            tc.tile_pool(name="zpool", bufs=3) as zpool,
            tc.tile_pool(name="ps", bufs=1, space="PSUM") as ps,
            tc.tile_pool(name="dram", bufs=1, space="DRAM") as dram,
        ):
            # ---- constants & weights (loaded once, reused across repeats)
            ident = sb.tile([128, 128], FP, tag="ident")
            make_identity(nc, ident)
            ones_col = sb.tile([128, 1], FP, tag="ones_col")
            nc.vector.memset(ones_col, 1.0)
            ones_row = sb.tile([1, TPC], FP, tag="ones_row")
            nc.vector.memset(ones_row, 1.0)
            eps_t = sb.tile([1, 1], FP, tag="eps")
            nc.vector.memset(eps_t, EPS)

            thw = sb.tile([128, HIGH // 128, LOW], FP, tag="thw")
            nc.sync.dma_start(thw[:], theta_w[:, :].rearrange(
                "(ko ki) a -> ki ko a", ki=128))
            thb = sb.tile([128, LOW // 128], FP, tag="thb")
            nc.sync.dma_start(thb[:], theta_b[:].rearrange(
                "(ko ki) -> ki ko", ki=128))
            gext = sb.tile([128, 3, LOW], FP, tag="gext")
            nc.sync.dma_start(gext[:, 0:2, :], g_w[:, :].rearrange(
                "(ko ki) a -> ki ko a", ki=128))
            nc.sync.dma_start(gext[0:1, 2, :], g_b[:][None, :])
            pext = sb.tile([128, 3, LOW], FP, tag="pext")
            nc.sync.dma_start(pext[:, 0:2, :], phi_w[:, :].rearrange(
                "(ko ki) a -> ki ko a", ki=128))
            nc.sync.dma_start(pext[0:1, 2, :], phi_b[:][None, :])
            ww = sb.tile([128, LOW // 128, HIGH], FP, tag="ww")
            nc.sync.dma_start(ww[:], w_w[:, :].rearrange(
                "(ko ki) h -> ki ko h", ki=128))
            gamma_r = sb.tile([1, HIGH], FP, tag="gamma_r")
            nc.sync.dma_start(gamma_r[:], bn_gamma[:][None, :])
            beta_r = sb.tile([1, HIGH], FP, tag="beta_r")
            nc.sync.dma_start(beta_r[:], bn_beta[:][None, :])

            for _ in range(repeats):
                # ---- x_l load (token-major) with homogeneous ones column
                xle = sb.tile([128, TT, LOWE], FP, tag="xle")
                nc.vector.memset(xle[:, :, LOW:LOWE], 1.0)
                for i in range(TT):
                    nc.sync.dma_start(xle[:, i, 0:LOW],
                                      x_l[i * 128:(i + 1) * 128, :])

                # ---- C_ext = X_ext^T X_ext  (257x257), contract over tokens
                cl = sb.tile([128, 3, LOWE], FP, tag="cl")
                for mc in range(3):
                    msl = _mslice(mc)
                    mlen = msl.stop - msl.start
                    cps_full = ps.tile([128, 512], FP, tag="mm", bufs=4)
                    cps = cps_full[:mlen, :LOWE]
                    for i in range(TT):
                        nc.tensor.matmul(cps, xle[:, i, msl], xle[:, i, :],
                                         start=(i == 0), stop=(i == TT - 1))
                    nc.vector.tensor_copy(cl[:mlen, mc, :], cps)

                # ---- AllReduce #1 (C_ext, 264KB)
                c_in = dram.tile([LOWE, LOWE], FP, tag="c_in")
                c_out = dram.tile([LOWE, LOWE], FP, tag="c_out")
                for mc in range(3):
                    msl = _mslice(mc)
                    mlen = msl.stop - msl.start
                    nc.sync.dma_start(c_in[msl, :], cl[:mlen, mc, :])
                nc.gpsimd.collective_compute(
                    "AllReduce", mybir.AluOpType.add, replica_groups=rg,
                    ins=[c_in.opt()], outs=[c_out.opt()])
                cg = sb.tile([128, 3, LOWE], FP, tag="cg")
                for mc in range(3):
                    msl = _mslice(mc)
                    mlen = msl.stop - msl.start
                    nc.sync.dma_start(cg[:mlen, mc, :], c_out[msl, :])

                # ---- x_h load + PE transpose (overlaps AR1)
                xh = sb.tile([128, TT, HIGH], FP, tag="xh")
                for i in range(TT):
                    nc.sync.dma_start(xh[:, i, :],
                                      x_h[i * 128:(i + 1) * 128, :])
                xht = sb.tile([128, HIGH // 128, TPC], FP, tag="xht")
                for i in range(TT):
                    for hc in range(HIGH // 128):
                        tp = ps.tile([128, 128], FP, tag="tp", bufs=2)
                        nc.tensor.transpose(
                            tp, xh[:, i, hc * 128:(hc + 1) * 128], ident)
                        nc.vector.tensor_copy(
                            xht[:, hc, i * 128:(i + 1) * 128], tp)

                # ---- thetaT = theta_w^T @ xh^T + theta_b  (feature-major)
                tht = sb.tile([128, LOW // 128, TPC], FP, tag="tht")
                for mc in range(LOW // 128):
                    for nn in range(TPC // 512):
                        tps = ps.tile([128, 512], FP, tag="mm", bufs=4)
                        for k in range(HIGH // 128):
                            nc.tensor.matmul(
                                tps, thw[:, k, mc * 128:(mc + 1) * 128],
                                xht[:, k, nn * 512:(nn + 1) * 512],
                                start=(k == 0), stop=(k == HIGH // 128 - 1))
                        nc.vector.tensor_scalar(
                            tht[:, mc, nn * 512:(nn + 1) * 512], tps,
                            thb[:, mc:mc + 1], None, mybir.AluOpType.add)

                # ---- post-AR1 chain: T1 = Cg @ Gext  (257x256)
                t1 = sb.tile([128, 3, LOW], FP, tag="t1")
                for mc in range(3):
                    msl = _mslice(mc)
                    mlen = msl.stop - msl.start
                    t1ps_full = ps.tile([128, 512], FP, tag="mm", bufs=4)
                    t1ps = t1ps_full[:mlen, :LOW]
                    for k in range(3):
                        klen = 128 if k < 2 else 1
                        nc.tensor.matmul(t1ps, cg[:klen, k, msl],
                                         gext[:klen, k, :],
                                         start=(k == 0), stop=(k == 2))
                    nc.vector.tensor_copy(t1[:mlen, mc, :], t1ps)

                # ---- MpT = (T1^T @ Pext) / BN   (M'^T, 256x256)
                mpt = sb.tile([128, LOW // 128, LOW], FP, tag="mpt")
                for bc in range(LOW // 128):
                    mps_full = ps.tile([128, 512], FP, tag="mm", bufs=4)
                    mps = mps_full[:, :LOW]
                    for k in range(3):
                        klen = 128 if k < 2 else 1
                        nc.tensor.matmul(
                            mps, t1[:klen, k, bc * 128:(bc + 1) * 128],
                            pext[:klen, k, :],
                            start=(k == 0), stop=(k == 2))
                    nc.vector.tensor_scalar_mul(mpt[:, bc, :], mps, 1.0 / BN)

                # ---- V = M' @ w_w   (256x512)
                v = sb.tile([128, LOW // 128, HIGH], FP, tag="v")
                for ac in range(LOW // 128):
                    vps = ps.tile([128, HIGH], FP, tag="mm", bufs=4)
                    for k in range(LOW // 128):
                        nc.tensor.matmul(
                            vps, mpt[:, k, ac * 128:(ac + 1) * 128],
                            ww[:, k, :], start=(k == 0),
                            stop=(k == LOW // 128 - 1))
                    nc.vector.tensor_copy(v[:, ac, :], vps)

                # ---- w_y' = theta_x @ V  (token-major) + BN stats
                wy = sb.tile([128, TT, HIGH], FP, tag="wy")
                sq = sb.tile([128, TT, HIGH], FP, tag="sq")
                sum_ps_full = ps.tile([128, HIGH], FP, tag="acc", bufs=2)
                sum_ps = sum_ps_full[0:1]
                ssq_ps_full = ps.tile([128, HIGH], FP, tag="acc", bufs=2)
                ssq_ps = ssq_ps_full[0:1]
                for i in range(TT):
                    wps = ps.tile([128, HIGH], FP, tag="mm", bufs=4)
                    for k in range(LOW // 128):
                        nc.tensor.matmul(
                            wps, tht[:, k, i * 128:(i + 1) * 128], v[:, k, :],
                            start=(k == 0), stop=(k == LOW // 128 - 1))
                    nc.vector.tensor_copy(wy[:, i, :], wps)
                    nc.scalar.activation(sq[:, i, :], wy[:, i, :],
                                         mybir.ActivationFunctionType.Square)
                    nc.tensor.matmul(sum_ps, ones_col, wy[:, i, :],
                                     start=(i == 0), stop=(i == TT - 1))
                    nc.tensor.matmul(ssq_ps, ones_col, sq[:, i, :],
                                     start=(i == 0), stop=(i == TT - 1))
                stats_sum = sb.tile([1, HIGH], FP, tag="stats_sum")
                stats_ssq = sb.tile([1, HIGH], FP, tag="stats_ssq")
                nc.vector.tensor_copy(stats_sum[:], sum_ps)
                nc.vector.tensor_copy(stats_ssq[:], ssq_ps)

                # ---- AllReduce #2 (stats, 4KB)
                s_in = dram.tile([2, HIGH], FP, tag="s_in")
                s_out = dram.tile([2, HIGH], FP, tag="s_out")
                nc.sync.dma_start(s_in[0:1, :], stats_sum[:])
                nc.sync.dma_start(s_in[1:2, :], stats_ssq[:])
                nc.gpsimd.collective_compute(
                    "AllReduce", mybir.AluOpType.add, replica_groups=rg,
                    ins=[s_in.opt()], outs=[s_out.opt()])
                sg_sum = sb.tile([1, HIGH], FP, tag="sg_sum")
                sg_ssq = sb.tile([1, HIGH], FP, tag="sg_ssq")
                nc.sync.dma_start(sg_sum[:], s_out[0:1, :])
                nc.sync.dma_start(sg_ssq[:], s_out[1:2, :])

                # ---- A = gamma * rsqrt(var+eps), B = beta - mean*A
                mean_r = sb.tile([1, HIGH], FP, tag="mean_r")
                nc.vector.tensor_scalar_mul(mean_r[:], sg_sum[:], 1.0 / BN)
                var_r = sb.tile([1, HIGH], FP, tag="var_r")
                # var = E[x^2] - mean^2
                nc.vector.tensor_scalar(var_r[:], sg_ssq[:], 1.0 / BN, None,
                                        mybir.AluOpType.mult)
                msq_r = sb.tile([1, HIGH], FP, tag="msq_r")
                nc.vector.tensor_mul(msq_r[:], mean_r[:], mean_r[:])
                nc.vector.tensor_sub(var_r[:], var_r[:], msq_r[:])
                std_r = sb.tile([1, HIGH], FP, tag="std_r")
                nc.scalar.activation(std_r[:], var_r[:],
                                     mybir.ActivationFunctionType.Sqrt,
                                     bias=eps_t[:])
                nc.vector.reciprocal(std_r[:], std_r[:])
                a_r = sb.tile([1, HIGH], FP, tag="a_r")
                nc.vector.tensor_mul(a_r[:], gamma_r[:], std_r[:])
                b_r = sb.tile([1, HIGH], FP, tag="b_r")
                nc.vector.tensor_mul(b_r[:], mean_r[:], a_r[:])
                nc.vector.tensor_sub(b_r[:], beta_r[:], b_r[:])

                # ---- broadcast A over 128 partitions via rank-1 matmul
                ab_ps = ps.tile([128, HIGH], FP, tag="acc", bufs=2)
                nc.tensor.matmul(ab_ps, ones_row[0:1, 0:128], a_r[:],
                                 start=True, stop=True)
                va = sb.tile([128, LOW // 128, HIGH], FP, tag="va")
                for ac in range(LOW // 128):
                    nc.vector.tensor_mul(va[:, ac, :], v[:, ac, :], ab_ps)

                # ---- z = theta_ext @ [V*A; B] + x_h
                for i in range(TT):
                    tsl = slice(i * 128, (i + 1) * 128)
                    zps = ps.tile([128, HIGH], FP, tag="mm", bufs=4)
                    nc.tensor.matmul(zps, tht[:, 0, tsl], va[:, 0, :],
                                     start=True, stop=False)
                    nc.tensor.matmul(zps, tht[:, 1, tsl], va[:, 1, :],
                                     start=False, stop=False)
                    nc.tensor.matmul(zps, ones_row[0:1, tsl], b_r[:],
                                     start=False, stop=True)
                    zt = zpool.tile([128, HIGH], FP, tag="z")
                    nc.vector.tensor_add(zt[:], zps, xh[:, i, :])
                    nc.sync.dma_start(z_out[tsl, :], zt[:])

    nc.compile()
    return nc


_CACHE: dict[int, "bacc.Bacc"] = {}
_LOCK = threading.Lock()


def _get_nc(repeats: int = 1):
    with _LOCK:
        if repeats not in _CACHE:
            _CACHE[repeats] = build_kernel(repeats)
        return _CACHE[repeats]


def _shard_inputs(inputs: dict) -> list[dict]:
    xh = np.ascontiguousarray(
        np.asarray(inputs["x_h"], dtype=np.float32).reshape(BN, HIGH))
    xl = np.ascontiguousarray(
        np.asarray(inputs["x_l"], dtype=np.float32).reshape(BN, LOW))
    # w_b is intentionally unused: BatchNorm output is invariant to a
    # constant shift of its input, so the w_b add cancels exactly.
    common = {
        "g_w": np.asarray(inputs["g_w"], np.float32),
        "g_b": np.asarray(inputs["g_b"], np.float32),
        "theta_w": np.asarray(inputs["theta_w"], np.float32),
        "theta_b": np.asarray(inputs["theta_b"], np.float32),
        "phi_w": np.asarray(inputs["phi_w"], np.float32),
        "phi_b": np.asarray(inputs["phi_b"], np.float32),
        "w_w": np.asarray(inputs["w_w"], np.float32),
        "bn_gamma": np.asarray(inputs["bn_gamma"], np.float32),
        "bn_beta": np.asarray(inputs["bn_beta"], np.float32),
    }
    return [
        {"x_h": xh[c * TPC:(c + 1) * TPC],
         "x_l": xl[c * TPC:(c + 1) * TPC], **common}
        for c in range(NCORES)
    ]


def kernel(**inputs) -> np.ndarray:
    nc = _get_nc(1)
    in_maps = _shard_inputs(inputs)
    res = run_bass_kernel_spmd(nc, in_maps, list(range(NCORES)))
    z = np.concatenate([res.results[c]["z"] for c in range(NCORES)], axis=0)
    return z.reshape(B, N, HIGH)


# revision 10
# speedup vs baseline: 1.9847x; 1.9847x over previous
"""MFA block kernel for 8 Trainium2 NeuronCores.

Full (unsharded) inputs in, full output out. Tokens (8*1024 = 8192) are
sharded across 8 cores (1024 each).  Uses the associative rewrite

    y = theta_x @ (phi_x^T @ g_x) / BN

so the (BN, BN) attention matrix is never formed.  With X_ext = [x_l | 1],

    M = phi_x^T g_x = P_ext^T (X_ext^T X_ext) G_ext,  P_ext = [phi_w; phi_b],
                                                      G_ext = [g_w; g_b],

so only C_ext = X_ext^T X_ext (257x257) needs an AllReduce.  C_ext is
symmetric, so the payload is triangle-packed as two rectangles (rows 0:128
x all cols, rows 128:256 x cols 128:257); the mirrored block and the s-row
are read back transposed directly from the DRAM bounce buffer.

BatchNorm (training mode) needs global per-feature sum / sum-of-squares of
w_y' = theta_x @ (M/BN) @ w_w  -- a second, tiny (2x512) AllReduce.  w_b is
dropped entirely: BN output is invariant to constant input shifts.

Everything on the x_h side lives feature-major ([feature, token]) so that
BN stats are free-dim reduces and the BN apply is a per-partition
tensor_scalar; x_h^T is loaded and z^T stored via transposed-AP DMAs.
"""

import threading

import numpy as np

import concourse.tile as tile
from concourse import bacc, mybir
from concourse.bass_utils import run_bass_kernel_spmd

FP = mybir.dt.float32
HIGH = 512
LOW = 256
B = 8
N = 1024
BN = B * N            # 8192 flattened tokens
NCORES = 8
TPC = BN // NCORES    # 1024 tokens per core
TT = TPC // 128       # 8 token tiles per core
HC = HIGH // 128      # 4 feature chunks of x_h / w_y / z
EPS = 1e-5

LOWE = LOW + 1        # 257: homogeneous low dim


def build_kernel(repeats: int = 1):
    nc = bacc.Bacc("TRN2", target_bir_lowering=False, debug=False,
                   num_devices=NCORES)

    x_h = nc.declare_dram_parameter("x_h", [TPC, HIGH], FP, isOutput=False)
    x_l = nc.declare_dram_parameter("x_l", [TPC, LOW], FP, isOutput=False)
    g_w = nc.declare_dram_parameter("g_w", [LOW, LOW], FP, isOutput=False)
    g_b = nc.declare_dram_parameter("g_b", [LOW], FP, isOutput=False)
    theta_w = nc.declare_dram_parameter("theta_w", [HIGH, LOW], FP, isOutput=False)
    theta_b = nc.declare_dram_parameter("theta_b", [LOW], FP, isOutput=False)
    phi_w = nc.declare_dram_parameter("phi_w", [LOW, LOW], FP, isOutput=False)
    phi_b = nc.declare_dram_parameter("phi_b", [LOW], FP, isOutput=False)
    w_w = nc.declare_dram_parameter("w_w", [LOW, HIGH], FP, isOutput=False)
    bn_gamma = nc.declare_dram_parameter("bn_gamma", [HIGH], FP, isOutput=False)
    bn_beta = nc.declare_dram_parameter("bn_beta", [HIGH], FP, isOutput=False)
    z_out = nc.declare_dram_parameter("z", [TPC, HIGH], FP, isOutput=True)

    rg = [list(range(NCORES))]

    with tile.TileContext(nc) as tc:
        with (
            tc.tile_pool(name="sb", bufs=1) as sb,
            tc.tile_pool(name="ps", bufs=1, space="PSUM") as ps,
            tc.tile_pool(name="dram", bufs=1, space="DRAM") as dram,
        ):
            # ---- constants & weights (loaded once)
            eps_c = sb.tile([128, 1], FP, tag="eps_c")
            nc.vector.memset(eps_c, EPS)
            thw = sb.tile([128, HIGH // 128, LOW], FP, tag="thw")
            nc.sync.dma_start(thw[:], theta_w[:, :].rearrange(
                "(ko ki) a -> ki ko a", ki=128))
            thb = sb.tile([128, LOW // 128], FP, tag="thb")
            nc.sync.dma_start(thb[:], theta_b[:].rearrange(
                "(ko ki) -> ki ko", ki=128))
            gext = sb.tile([128, 3, LOW], FP, tag="gext")
            nc.sync.dma_start(gext[:, 0:2, :], g_w[:, :].rearrange(
                "(ko ki) a -> ki ko a", ki=128))
            nc.sync.dma_start(gext[0:1, 2, :], g_b[:][None, :])
            pext = sb.tile([128, 3, LOW], FP, tag="pext")
            nc.sync.dma_start(pext[:, 0:2, :], phi_w[:, :].rearrange(
                "(ko ki) a -> ki ko a", ki=128))
            nc.sync.dma_start(pext[0:1, 2, :], phi_b[:][None, :])
            ww = sb.tile([128, LOW // 128, HIGH], FP, tag="ww")
            nc.sync.dma_start(ww[:], w_w[:, :].rearrange(
                "(ko ki) h -> ki ko h", ki=128))
            gamma_p = sb.tile([128, HC], FP, tag="gamma_p")
            nc.sync.dma_start(gamma_p[:], bn_gamma[:].rearrange(
                "(hc p) -> p hc", p=128))
            beta_p = sb.tile([128, HC], FP, tag="beta_p")
            nc.sync.dma_start(beta_p[:], bn_beta[:].rearrange(
                "(hc p) -> p hc", p=128))

            for _ in range(repeats):
                # ---- x_l load (token-major) + homogeneous ones column
                xle = sb.tile([128, TT, LOWE], FP, tag="xle")
                nc.vector.memset(xle[:, :, LOW:LOWE], 1.0)
                for i in range(TT):
                    nc.sync.dma_start(xle[:, i, 0:LOW],
                                      x_l[i * 128:(i + 1) * 128, :])

                # ---- x_h^T via transposed-AP DMA loads (feature-major)
                xht = sb.tile([128, HC, TPC], FP, tag="xht")
                with nc.allow_non_contiguous_dma(reason="transposed x_h load"):
                    for hc in range(HC):
                        nc.sync.dma_start(
                            xht[:, hc, :],
                            x_h[:, hc * 128:(hc + 1) * 128].rearrange(
                                "t p -> p t"))

                # ---- C_rect = X_ext[:, 0:256]^T @ X_ext  (256 x 257)
                cl = sb.tile([128, 2, LOWE], FP, tag="cl")
                for mc in range(2):
                    cps = ps.tile([128, 512], FP, tag="mm", bufs=4)
                    for i in range(TT):
                        nc.tensor.matmul(
                            cps[:, :LOWE],
                            xle[:, i, mc * 128:(mc + 1) * 128],
                            xle[:, i, :],
                            start=(i == 0), stop=(i == TT - 1))
                    nc.vector.tensor_copy(cl[:, mc, :], cps[:, :LOWE])

                # ---- AllReduce #1: triangle-packed C (198KB)
                # c_in[:, 0:257]  = C rows 0:128, all cols (incl. s col)
                # c_in[:, 257:386] = C rows 128:256, cols 128:257
                c_in = dram.tile([128, 386], FP, tag="c_in")
                c_out = dram.tile([128, 386], FP, tag="c_out")
                nc.sync.dma_start(c_in[:, 0:LOWE], cl[:, 0, :])
                nc.sync.dma_start(c_in[:, LOWE:386], cl[:, 1, 128:LOWE])
                nc.gpsimd.collective_compute(
                    "AllReduce", mybir.AluOpType.add, replica_groups=rg,
                    ins=[c_in.opt()], outs=[c_out.opt()])
                # Reconstruct the three k-tiles of C_ext from the bounce:
                # cga = rows 0:128 (all 257 cols);  cgb = rows 128:256:
                #   cols 0:128 mirrored from block(0,1)^T, cols 128:257 direct.
                # srow = s^T (row 256) from the s columns; corner = BN.
                cga = sb.tile([128, LOWE], FP, tag="cga")
                nc.sync.dma_start(cga[:], c_out[:, 0:LOWE])
                cgb = sb.tile([128, LOWE], FP, tag="cgb")
                nc.sync.dma_start(cgb[:, 128:LOWE], c_out[:, LOWE:386])
                srow = sb.tile([1, LOWE], FP, tag="srow")
                with nc.allow_non_contiguous_dma(reason="transposed C read"):
                    nc.sync.dma_start(
                        cgb[:, 0:128],
                        c_out[:, 128:256].rearrange("p q -> q p"))
                    nc.sync.dma_start(
                        srow[:, 0:128],
                        c_out[:, 256:LOWE].rearrange("p o -> o p"))
                    nc.sync.dma_start(
                        srow[:, 128:256],
                        c_out[:, 385:386].rearrange("p o -> o p"))
                nc.vector.memset(srow[:, 256:LOWE], float(BN))

                # ---- thetaT = theta_w^T @ x_h^T + theta_b  (feature-major;
                #      overlaps AR1)
                tht = sb.tile([128, LOW // 128, TPC], FP, tag="tht")
                for mc in range(LOW // 128):
                    for nn in range(TPC // 512):
                        tps = ps.tile([128, 512], FP, tag="mm", bufs=4)
                        for k in range(HIGH // 128):
                            nc.tensor.matmul(
                                tps, thw[:, k, mc * 128:(mc + 1) * 128],
                                xht[:, k, nn * 512:(nn + 1) * 512],
                                start=(k == 0), stop=(k == HIGH // 128 - 1))
                        nc.vector.tensor_scalar(
                            tht[:, mc, nn * 512:(nn + 1) * 512], tps,
                            thb[:, mc:mc + 1], None, mybir.AluOpType.add)

                # ---- T1 = C_ext @ G_ext  (257 x 256)
                cg_tiles = [cga, cgb, srow]
                t1 = sb.tile([128, 3, LOW], FP, tag="t1")
                for mc in range(3):
                    msl = (slice(0, 128), slice(128, 256),
                           slice(256, 257))[mc]
                    mlen = msl.stop - msl.start
                    t1f = ps.tile([128, 512], FP, tag="mm", bufs=4)
                    t1ps = t1f[:mlen, :LOW]
                    for k in range(3):
                        klen = 128 if k < 2 else 1
                        nc.tensor.matmul(t1ps, cg_tiles[k][:klen, msl],
                                         gext[:klen, k, :],
                                         start=(k == 0), stop=(k == 2))
                    nc.vector.tensor_copy(t1[:mlen, mc, :], t1ps)

                # ---- MpT = (T1^T @ P_ext) / BN   (M'^T, 256 x 256)
                mpt = sb.tile([128, LOW // 128, LOW], FP, tag="mpt")
                for bc in range(LOW // 128):
                    mpf = ps.tile([128, 512], FP, tag="mm", bufs=4)
                    mps = mpf[:, :LOW]
                    for k in range(3):
                        klen = 128 if k < 2 else 1
                        nc.tensor.matmul(
                            mps, t1[:klen, k, bc * 128:(bc + 1) * 128],
                            pext[:klen, k, :],
                            start=(k == 0), stop=(k == 2))
                    nc.vector.tensor_scalar_mul(mpt[:, bc, :], mps, 1.0 / BN)

                # ---- V = M' @ w_w   (256 x 512)
                v = sb.tile([128, LOW // 128, HIGH], FP, tag="v")
                for ac in range(LOW // 128):
                    vps = ps.tile([128, 512], FP, tag="mm", bufs=4)
                    for k in range(LOW // 128):
                        nc.tensor.matmul(
                            vps, mpt[:, k, ac * 128:(ac + 1) * 128],
                            ww[:, k, :], start=(k == 0),
                            stop=(k == LOW // 128 - 1))
                    nc.vector.tensor_copy(v[:, ac, :], vps)

                # ---- w_y'^T = V^T-chunks @ thetaT   (feature-major)
                wyt = sb.tile([128, HC, TPC], FP, tag="wyt")
                for hc in range(HC):
                    for nn in range(TPC // 512):
                        wps = ps.tile([128, 512], FP, tag="mm", bufs=4)
                        for k in range(LOW // 128):
                            nc.tensor.matmul(
                                wps, v[:, k, hc * 128:(hc + 1) * 128],
                                tht[:, k, nn * 512:(nn + 1) * 512],
                                start=(k == 0), stop=(k == LOW // 128 - 1))
                        nc.vector.tensor_copy(
                            wyt[:, hc, nn * 512:(nn + 1) * 512], wps)

                # ---- BN stats: per-partition free-dim reduces
                sqt = sb.tile([128, HC, TPC], FP, tag="sqt")
                nc.scalar.activation(sqt[:], wyt[:],
                                     mybir.ActivationFunctionType.Square)
                ssum = sb.tile([128, HC], FP, tag="ssum")
                nc.vector.reduce_sum(ssum[:], wyt[:],
                                     axis=mybir.AxisListType.X)
                ssq = sb.tile([128, HC], FP, tag="ssq")
                nc.vector.reduce_sum(ssq[:], sqt[:],
                                     axis=mybir.AxisListType.X)

                # ---- AllReduce #2 (stats, 4KB)
                s_in = dram.tile([2, HIGH], FP, tag="s_in")
                s_out = dram.tile([2, HIGH], FP, tag="s_out")
                nc.sync.dma_start(
                    s_in[0, :].rearrange("(hc p) -> p hc", p=128), ssum[:])
                nc.sync.dma_start(
                    s_in[1, :].rearrange("(hc p) -> p hc", p=128), ssq[:])
                nc.gpsimd.collective_compute(
                    "AllReduce", mybir.AluOpType.add, replica_groups=rg,
                    ins=[s_in.opt()], outs=[s_out.opt()])
                sgs = sb.tile([128, HC], FP, tag="sgs")
                nc.sync.dma_start(
                    sgs[:], s_out[0, :].rearrange("(hc p) -> p hc", p=128))
                sgq = sb.tile([128, HC], FP, tag="sgq")
                nc.sync.dma_start(
                    sgq[:], s_out[1, :].rearrange("(hc p) -> p hc", p=128))

                # ---- r = x_h^T + beta  (independent of AR2 -> overlaps it)
                r_t = sb.tile([128, HC, TPC], FP, tag="r_t")
                for hc in range(HC):
                    nc.vector.tensor_scalar(
                        r_t[:, hc, :], xht[:, hc, :], beta_p[:, hc:hc + 1],
                        None, mybir.AluOpType.add)

                # ---- A = gamma * rsqrt(var+eps); all per-partition [128, HC]
                mean_p = sb.tile([128, HC], FP, tag="mean_p")
                nc.vector.tensor_scalar_mul(mean_p[:], sgs[:], 1.0 / BN)
                ex2_p = sb.tile([128, HC], FP, tag="ex2_p")
                nc.vector.tensor_scalar_mul(ex2_p[:], sgq[:], 1.0 / BN)
                msq_p = sb.tile([128, HC], FP, tag="msq_p")
                nc.vector.tensor_mul(msq_p[:], mean_p[:], mean_p[:])
                var_p = sb.tile([128, HC], FP, tag="var_p")
                nc.vector.tensor_sub(var_p[:], ex2_p[:], msq_p[:])
                std_p = sb.tile([128, HC], FP, tag="std_p")
                nc.scalar.activation(std_p[:], var_p[:],
                                     mybir.ActivationFunctionType.Sqrt,
                                     bias=eps_c[:])
                nc.vector.reciprocal(std_p[:], std_p[:])
                a_p = sb.tile([128, HC], FP, tag="a_p")
                nc.vector.tensor_mul(a_p[:], gamma_p[:], std_p[:])

                # ---- z^T = (w_y'^T - mean)*A + (x_h^T + beta); store
                zt = sb.tile([128, HC, TPC], FP, tag="zt")
                with nc.allow_non_contiguous_dma(reason="transposed z store"):
                    for hc in range(HC):
                        nc.vector.tensor_scalar(
                            zt[:, hc, :], wyt[:, hc, :],
                            mean_p[:, hc:hc + 1], a_p[:, hc:hc + 1],
                            mybir.AluOpType.subtract, mybir.AluOpType.mult)
                        nc.vector.tensor_add(zt[:, hc, :], zt[:, hc, :],
                                             r_t[:, hc, :])
                        nc.sync.dma_start(
                            z_out[:, hc * 128:(hc + 1) * 128].rearrange(
                                "t p -> p t"),
                            zt[:, hc, :])

    nc.compile()
    return nc


_CACHE: dict[int, "bacc.Bacc"] = {}
_LOCK = threading.Lock()


def _get_nc(repeats: int = 1):
    with _LOCK:
        if repeats not in _CACHE:
            _CACHE[repeats] = build_kernel(repeats)
        return _CACHE[repeats]


def _shard_inputs(inputs: dict) -> list[dict]:
    xh = np.ascontiguousarray(
        np.asarray(inputs["x_h"], dtype=np.float32).reshape(BN, HIGH))
    xl = np.ascontiguousarray(
        np.asarray(inputs["x_l"], dtype=np.float32).reshape(BN, LOW))
    # w_b is intentionally unused: BatchNorm output is invariant to a
    # constant shift of its input, so the w_b add cancels exactly.
    common = {
        "g_w": np.asarray(inputs["g_w"], np.float32),
        "g_b": np.asarray(inputs["g_b"], np.float32),
        "theta_w": np.asarray(inputs["theta_w"], np.float32),
        "theta_b": np.asarray(inputs["theta_b"], np.float32),
        "phi_w": np.asarray(inputs["phi_w"], np.float32),
        "phi_b": np.asarray(inputs["phi_b"], np.float32),
        "w_w": np.asarray(inputs["w_w"], np.float32),
        "bn_gamma": np.asarray(inputs["bn_gamma"], np.float32),
        "bn_beta": np.asarray(inputs["bn_beta"], np.float32),
    }
    return [
        {"x_h": xh[c * TPC:(c + 1) * TPC],
         "x_l": xl[c * TPC:(c + 1) * TPC], **common}
        for c in range(NCORES)
    ]


def kernel(**inputs) -> np.ndarray:
    nc = _get_nc(1)
    in_maps = _shard_inputs(inputs)
    res = run_bass_kernel_spmd(nc, in_maps, list(range(NCORES)))
    z = np.concatenate([res.results[c]["z"] for c in range(NCORES)], axis=0)
    return z.reshape(B, N, HIGH)


# revision 12
# speedup vs baseline: 1.9955x; 1.0055x over previous
"""MFA block kernel for 8 Trainium2 NeuronCores.

Full (unsharded) inputs in, full output out. Tokens (8*1024 = 8192) are
sharded across 8 cores (1024 each).  Uses the associative rewrite

    y = theta_x @ (phi_x^T @ g_x) / BN

so the (BN, BN) attention matrix is never formed.  With X_ext = [x_l | 1],

    M = phi_x^T g_x = P_ext^T (X_ext^T X_ext) G_ext,  P_ext = [phi_w; phi_b],
                                                      G_ext = [g_w; g_b],

so only C_ext = X_ext^T X_ext (257x257) needs an AllReduce.  C_ext is
symmetric, so the payload is triangle-packed as two rectangles (rows 0:128
x all cols, rows 128:256 x cols 128:257); the mirrored block and the s-row
are read back transposed directly from the DRAM bounce buffer.

BatchNorm (training mode) needs global per-feature sum / sum-of-squares of
w_y' = theta_x @ (M/BN) @ w_w  -- a second, tiny (2x512) AllReduce.  w_b is
dropped entirely: BN output is invariant to constant input shifts.

Everything on the x_h side lives feature-major ([feature, token]) so that
BN stats are free-dim reduces and the BN apply is a per-partition
tensor_scalar; x_h^T is loaded and z^T stored via transposed-AP DMAs.
"""

import threading

import numpy as np

import concourse.tile as tile
from concourse import bacc, mybir
from concourse.bass_utils import run_bass_kernel_spmd

FP = mybir.dt.float32
HIGH = 512
LOW = 256
B = 8
N = 1024
BN = B * N            # 8192 flattened tokens
NCORES = 8
TPC = BN // NCORES    # 1024 tokens per core
TT = TPC // 128       # 8 token tiles per core
HC = HIGH // 128      # 4 feature chunks of x_h / w_y / z
EPS = 1e-5

LOWE = LOW + 1        # 257: homogeneous low dim


def build_kernel(repeats: int = 1, noar: bool = False):
    nc = bacc.Bacc("TRN2", target_bir_lowering=False, debug=False,
                   num_devices=NCORES)

    x_h = nc.declare_dram_parameter("x_h", [TPC, HIGH], FP, isOutput=False)
    x_l = nc.declare_dram_parameter("x_l", [TPC, LOW], FP, isOutput=False)
    g_w = nc.declare_dram_parameter("g_w", [LOW, LOW], FP, isOutput=False)
    g_b = nc.declare_dram_parameter("g_b", [LOW], FP, isOutput=False)
    theta_w = nc.declare_dram_parameter("theta_w", [HIGH, LOW], FP, isOutput=False)
    theta_b = nc.declare_dram_parameter("theta_b", [LOW], FP, isOutput=False)
    phi_w = nc.declare_dram_parameter("phi_w", [LOW, LOW], FP, isOutput=False)
    phi_b = nc.declare_dram_parameter("phi_b", [LOW], FP, isOutput=False)
    w_w = nc.declare_dram_parameter("w_w", [LOW, HIGH], FP, isOutput=False)
    bn_gamma = nc.declare_dram_parameter("bn_gamma", [HIGH], FP, isOutput=False)
    bn_beta = nc.declare_dram_parameter("bn_beta", [HIGH], FP, isOutput=False)
    z_out = nc.declare_dram_parameter("z", [TPC, HIGH], FP, isOutput=True)

    rg = [list(range(NCORES))]

    with tile.TileContext(nc) as tc:
        with (
            tc.tile_pool(name="sb", bufs=1) as sb,
            tc.tile_pool(name="ps", bufs=1, space="PSUM") as ps,
            tc.tile_pool(name="dram", bufs=1, space="DRAM") as dram,
        ):
            # ---- small constants (chain weights load later, after the
            #      input DMAs, so inputs win the DMA queues)
            eps_c = sb.tile([128, 1], FP, tag="eps_c")
            nc.vector.memset(eps_c, EPS)

            for _ in range(repeats):
                # ---- x_l load (token-major) + homogeneous ones column
                xle = sb.tile([128, TT, LOWE], FP, tag="xle")
                nc.vector.memset(xle[:, :, LOW:LOWE], 1.0)
                for i in range(TT):
                    nc.sync.dma_start(xle[:, i, 0:LOW],
                                      x_l[i * 128:(i + 1) * 128, :])

                # ---- x_h^T via transposed-AP DMA loads (feature-major)
                xht = sb.tile([128, HC, TPC], FP, tag="xht")
                with nc.allow_non_contiguous_dma(reason="transposed x_h load"):
                    for hc in range(HC):
                        nc.sync.dma_start(
                            xht[:, hc, :],
                            x_h[:, hc * 128:(hc + 1) * 128].rearrange(
                                "t p -> p t"))

                # ---- weights: thw/thb feed thetaT (runs under AR1);
                #      gext/pext/ww/gamma/beta are only needed post-AR1.
                thw = sb.tile([128, HIGH // 128, LOW], FP, tag="thw")
                nc.sync.dma_start(thw[:], theta_w[:, :].rearrange(
                    "(ko ki) a -> ki ko a", ki=128))
                thb = sb.tile([128, LOW // 128], FP, tag="thb")
                nc.sync.dma_start(thb[:], theta_b[:].rearrange(
                    "(ko ki) -> ki ko", ki=128))
                gext = sb.tile([128, 3, LOW], FP, tag="gext")
                nc.sync.dma_start(gext[:, 0:2, :], g_w[:, :].rearrange(
                    "(ko ki) a -> ki ko a", ki=128))
                nc.sync.dma_start(gext[0:1, 2, :], g_b[:][None, :])
                pext = sb.tile([128, 3, LOW], FP, tag="pext")
                nc.sync.dma_start(pext[:, 0:2, :], phi_w[:, :].rearrange(
                    "(ko ki) a -> ki ko a", ki=128))
                nc.sync.dma_start(pext[0:1, 2, :], phi_b[:][None, :])
                ww = sb.tile([128, LOW // 128, HIGH], FP, tag="ww")
                nc.sync.dma_start(ww[:], w_w[:, :].rearrange(
                    "(ko ki) h -> ki ko h", ki=128))
                gamma_p = sb.tile([128, HC], FP, tag="gamma_p")
                nc.sync.dma_start(gamma_p[:], bn_gamma[:].rearrange(
                    "(hc p) -> p hc", p=128))
                beta_p = sb.tile([128, HC], FP, tag="beta_p")
                nc.sync.dma_start(beta_p[:], bn_beta[:].rearrange(
                    "(hc p) -> p hc", p=128))

                # ---- C_rect = X_ext[:, 0:256]^T @ X_ext  (256 x 257)
                cl = sb.tile([128, 2, LOWE], FP, tag="cl")
                for mc in range(2):
                    cps = ps.tile([128, 512], FP, tag="mm", bufs=4)
                    for i in range(TT):
                        nc.tensor.matmul(
                            cps[:, :LOWE],
                            xle[:, i, mc * 128:(mc + 1) * 128],
                            xle[:, i, :],
                            start=(i == 0), stop=(i == TT - 1))
                    nc.vector.tensor_copy(cl[:, mc, :], cps[:, :LOWE])

                # ---- AllReduce #1: triangle-packed C (198KB)
                # c_in[:, 0:257]  = C rows 0:128, all cols (incl. s col)
                # c_in[:, 257:386] = C rows 128:256, cols 128:257
                c_in = dram.tile([128, 386], FP, tag="c_in")
                c_out = dram.tile([128, 386], FP, tag="c_out")
                nc.sync.dma_start(c_in[:, 0:LOWE], cl[:, 0, :])
                nc.sync.dma_start(c_in[:, LOWE:386], cl[:, 1, 128:LOWE])
                if noar:
                    nc.sync.dma_start(c_out[:, :], c_in[:, :])
                else:
                    nc.gpsimd.collective_compute(
                        "AllReduce", mybir.AluOpType.add, replica_groups=rg,
                        ins=[c_in.opt()], outs=[c_out.opt()])
                # Reconstruct the three k-tiles of C_ext from the bounce:
                # cga = rows 0:128 (all 257 cols);  cgb = rows 128:256:
                #   cols 0:128 mirrored from block(0,1)^T, cols 128:257 direct.
                # srow = s^T (row 256) from the s columns; corner = BN.
                cga = sb.tile([128, LOWE], FP, tag="cga")
                nc.sync.dma_start(cga[:], c_out[:, 0:LOWE])
                cgb = sb.tile([128, LOWE], FP, tag="cgb")
                nc.sync.dma_start(cgb[:, 128:LOWE], c_out[:, LOWE:386])
                srow = sb.tile([1, LOWE], FP, tag="srow")
                with nc.allow_non_contiguous_dma(reason="transposed C read"):
                    nc.sync.dma_start(
                        cgb[:, 0:128],
                        c_out[:, 128:256].rearrange("p q -> q p"))
                    nc.sync.dma_start(
                        srow[:, 0:128],
                        c_out[:, 256:LOWE].rearrange("p o -> o p"))
                    nc.sync.dma_start(
                        srow[:, 128:256],
                        c_out[:, 385:386].rearrange("p o -> o p"))
                nc.vector.memset(srow[:, 256:LOWE], float(BN))

                # ---- thetaT = theta_w^T @ x_h^T + theta_b  (feature-major;
                #      overlaps AR1)
                tht = sb.tile([128, LOW // 128, TPC], FP, tag="tht")
                for mc in range(LOW // 128):
                    for nn in range(TPC // 512):
                        tps = ps.tile([128, 512], FP, tag="mm", bufs=4)
                        for k in range(HIGH // 128):
                            nc.tensor.matmul(
                                tps, thw[:, k, mc * 128:(mc + 1) * 128],
                                xht[:, k, nn * 512:(nn + 1) * 512],
                                start=(k == 0), stop=(k == HIGH // 128 - 1))
                        nc.vector.tensor_scalar(
                            tht[:, mc, nn * 512:(nn + 1) * 512], tps,
                            thb[:, mc:mc + 1], None, mybir.AluOpType.add)

                # ---- T1 = C_ext @ G_ext  (257 x 256)
                cg_tiles = [cga, cgb, srow]
                t1 = sb.tile([128, 3, LOW], FP, tag="t1")
                for mc in range(3):
                    msl = (slice(0, 128), slice(128, 256),
                           slice(256, 257))[mc]
                    mlen = msl.stop - msl.start
                    t1f = ps.tile([128, 512], FP, tag="mm", bufs=4)
                    t1ps = t1f[:mlen, :LOW]
                    for k in range(3):
                        klen = 128 if k < 2 else 1
                        nc.tensor.matmul(t1ps, cg_tiles[k][:klen, msl],
                                         gext[:klen, k, :],
                                         start=(k == 0), stop=(k == 2))
                    nc.vector.tensor_copy(t1[:mlen, mc, :], t1ps)

                # ---- MpT = (T1^T @ P_ext) / BN   (M'^T, 256 x 256)
                mpt = sb.tile([128, LOW // 128, LOW], FP, tag="mpt")
                for bc in range(LOW // 128):
                    mpf = ps.tile([128, 512], FP, tag="mm", bufs=4)
                    mps = mpf[:, :LOW]
                    for k in range(3):
                        klen = 128 if k < 2 else 1
                        nc.tensor.matmul(
                            mps, t1[:klen, k, bc * 128:(bc + 1) * 128],
                            pext[:klen, k, :],
                            start=(k == 0), stop=(k == 2))
                    nc.vector.tensor_scalar_mul(mpt[:, bc, :], mps, 1.0 / BN)

                # ---- V = M' @ w_w   (256 x 512)
                v = sb.tile([128, LOW // 128, HIGH], FP, tag="v")
                for ac in range(LOW // 128):
                    vps = ps.tile([128, 512], FP, tag="mm", bufs=4)
                    for k in range(LOW // 128):
                        nc.tensor.matmul(
                            vps, mpt[:, k, ac * 128:(ac + 1) * 128],
                            ww[:, k, :], start=(k == 0),
                            stop=(k == LOW // 128 - 1))
                    nc.vector.tensor_copy(v[:, ac, :], vps)

                # ---- w_y'^T = V^T-chunks @ thetaT   (feature-major)
                wyt = sb.tile([128, HC, TPC], FP, tag="wyt")
                for hc in range(HC):
                    for nn in range(TPC // 512):
                        wps = ps.tile([128, 512], FP, tag="mm", bufs=4)
                        for k in range(LOW // 128):
                            nc.tensor.matmul(
                                wps, v[:, k, hc * 128:(hc + 1) * 128],
                                tht[:, k, nn * 512:(nn + 1) * 512],
                                start=(k == 0), stop=(k == LOW // 128 - 1))
                        nc.vector.tensor_copy(
                            wyt[:, hc, nn * 512:(nn + 1) * 512], wps)

                # ---- BN stats: per-partition free-dim reduces
                sqt = sb.tile([128, HC, TPC], FP, tag="sqt")
                nc.scalar.activation(sqt[:], wyt[:],
                                     mybir.ActivationFunctionType.Square)
                ssum = sb.tile([128, HC], FP, tag="ssum")
                nc.vector.reduce_sum(ssum[:], wyt[:],
                                     axis=mybir.AxisListType.X)
                ssq = sb.tile([128, HC], FP, tag="ssq")
                nc.vector.reduce_sum(ssq[:], sqt[:],
                                     axis=mybir.AxisListType.X)

                # ---- AllReduce #2 (stats, 4KB)
                s_in = dram.tile([2, HIGH], FP, tag="s_in")
                s_out = dram.tile([2, HIGH], FP, tag="s_out")
                nc.sync.dma_start(
                    s_in[0, :].rearrange("(hc p) -> p hc", p=128), ssum[:])
                nc.sync.dma_start(
                    s_in[1, :].rearrange("(hc p) -> p hc", p=128), ssq[:])
                if noar:
                    nc.sync.dma_start(s_out[:, :], s_in[:, :])
                else:
                    nc.gpsimd.collective_compute(
                        "AllReduce", mybir.AluOpType.add, replica_groups=rg,
                        ins=[s_in.opt()], outs=[s_out.opt()])
                sgs = sb.tile([128, HC], FP, tag="sgs")
                nc.sync.dma_start(
                    sgs[:], s_out[0, :].rearrange("(hc p) -> p hc", p=128))
                sgq = sb.tile([128, HC], FP, tag="sgq")
                nc.sync.dma_start(
                    sgq[:], s_out[1, :].rearrange("(hc p) -> p hc", p=128))

                # ---- r = x_h^T + beta  (independent of AR2 -> overlaps it)
                r_t = sb.tile([128, HC, TPC], FP, tag="r_t")
                for hc in range(HC):
                    nc.vector.tensor_scalar(
                        r_t[:, hc, :], xht[:, hc, :], beta_p[:, hc:hc + 1],
                        None, mybir.AluOpType.add)

                # ---- A = gamma * rsqrt(var+eps); all per-partition [128, HC]
                mean_p = sb.tile([128, HC], FP, tag="mean_p")
                nc.vector.tensor_scalar_mul(mean_p[:], sgs[:], 1.0 / BN)
                ex2_p = sb.tile([128, HC], FP, tag="ex2_p")
                nc.vector.tensor_scalar_mul(ex2_p[:], sgq[:], 1.0 / BN)
                msq_p = sb.tile([128, HC], FP, tag="msq_p")
                nc.vector.tensor_mul(msq_p[:], mean_p[:], mean_p[:])
                var_p = sb.tile([128, HC], FP, tag="var_p")
                nc.vector.tensor_sub(var_p[:], ex2_p[:], msq_p[:])
                std_p = sb.tile([128, HC], FP, tag="std_p")
                nc.scalar.activation(std_p[:], var_p[:],
                                     mybir.ActivationFunctionType.Sqrt,
                                     bias=eps_c[:])
                nc.vector.reciprocal(std_p[:], std_p[:])
                a_p = sb.tile([128, HC], FP, tag="a_p")
                nc.vector.tensor_mul(a_p[:], gamma_p[:], std_p[:])

                # ---- z^T = (w_y'^T - mean)*A + (x_h^T + beta); store
                zt = sb.tile([128, HC, TPC], FP, tag="zt")
                with nc.allow_non_contiguous_dma(reason="transposed z store"):
                    for hc in range(HC):
                        nc.vector.tensor_scalar(
                            zt[:, hc, :], wyt[:, hc, :],
                            mean_p[:, hc:hc + 1], a_p[:, hc:hc + 1],
                            mybir.AluOpType.subtract, mybir.AluOpType.mult)
                        nc.vector.tensor_add(zt[:, hc, :], zt[:, hc, :],
                                             r_t[:, hc, :])
                        nc.sync.dma_start(
                            z_out[:, hc * 128:(hc + 1) * 128].rearrange(
                                "t p -> p t"),
                            zt[:, hc, :])

    nc.compile()
    return nc


_CACHE: dict[int, "bacc.Bacc"] = {}
_LOCK = threading.Lock()


def _get_nc(repeats: int = 1):
    with _LOCK:
        if repeats not in _CACHE:
            _CACHE[repeats] = build_kernel(repeats)
        return _CACHE[repeats]


def _shard_inputs(inputs: dict) -> list[dict]:
    xh = np.ascontiguousarray(
        np.asarray(inputs["x_h"], dtype=np.float32).reshape(BN, HIGH))
    xl = np.ascontiguousarray(
        np.asarray(inputs["x_l"], dtype=np.float32).reshape(BN, LOW))
    # w_b is intentionally unused: BatchNorm output is invariant to a
    # constant shift of its input, so the w_b add cancels exactly.
    common = {
        "g_w": np.asarray(inputs["g_w"], np.float32),
        "g_b": np.asarray(inputs["g_b"], np.float32),
        "theta_w": np.asarray(inputs["theta_w"], np.float32),
        "theta_b": np.asarray(inputs["theta_b"], np.float32),
        "phi_w": np.asarray(inputs["phi_w"], np.float32),
        "phi_b": np.asarray(inputs["phi_b"], np.float32),
        "w_w": np.asarray(inputs["w_w"], np.float32),
        "bn_gamma": np.asarray(inputs["bn_gamma"], np.float32),
        "bn_beta": np.asarray(inputs["bn_beta"], np.float32),
    }
    return [
        {"x_h": xh[c * TPC:(c + 1) * TPC],
         "x_l": xl[c * TPC:(c + 1) * TPC], **common}
        for c in range(NCORES)
    ]


def kernel(**inputs) -> np.ndarray:
    nc = _get_nc(1)
    in_maps = _shard_inputs(inputs)
    res = run_bass_kernel_spmd(nc, in_maps, list(range(NCORES)))
    z = np.concatenate([res.results[c]["z"] for c in range(NCORES)], axis=0)
    return z.reshape(B, N, HIGH)
